# revision 1
# baseline (speedup 1.0000x reference)
"""GCN layer (normalized adjacency aggregation) on 8 Trainium2 NeuronCores.

Algorithm (row-sharded, minimal communication):
    a_hat = A + I  (identity folded into the shard on the host)
    deg[i] = sum_j a_hat[i, j]              -> per-core PE ones-matmul over its rows
    dinv = deg ** -0.5                      -> sqrt + reciprocal, AllGather (4KB)
    sup = x @ W.T + b                       -> computed redundantly per core (tiny)
    S = dinv[:, None] * sup
    out[i, :] = dinv[i] * (a_hat[i, :] @ S) -> accumulating matmul over the row block

Each core receives its row block of a_hat TRANSPOSED ([N, N/8], contraction dim
on partitions) so both the degree pass and the aggregation pass stream it as the
matmul moving operand with no on-chip transposes.  The block is streamed twice
(degree, then aggregation); the last NRES tiles of the first pass stay resident
in SBUF and are not re-read.  Matmuls run in float32r mode (single-pass fp32
multiply, 4x the instruction rate of full fp32 LOW_HIGH).
"""

import numpy as np
from contextlib import ExitStack

N = 8192
F = 128
NCORES = 8
RPC = N // NCORES  # 1024 rows per core
P = 128            # partitions
JT = N // P        # 64 column tiles of the (transposed) block

# SBUF knobs.  Per-partition budget ~192KB: sup_all 32KB + misc ~16KB +
# resident 10 pairs x 8KB + 3-pair phase-A stream + 4-pair reload pool.
NRES_PAIRS = 11
STREAM_PAIRS = 3
RELOAD_PAIRS = 3


def build_module(n=N, f=F, ncores=NCORES, nres_pairs=NRES_PAIRS,
                 stream_pairs=STREAM_PAIRS, reload_pairs=None,
                 use_f32r=True, debug_taps=False):
    """Build and compile the SPMD Bass module (same program on every core)."""
    from concourse import bass, bacc, tile

    mybir = bass.mybir
    dt = mybir.dt.float32
    dtr = mybir.dt.float32r if use_f32r else dt

    if reload_pairs is None:
        reload_pairs = RELOAD_PAIRS
    rpc = n // ncores
    jt = n // P
    pairs = jt // 2
    j_stream_pairs = pairs - nres_pairs
    j_stream = 2 * j_stream_pairs
    per_rank = rpc // P

    nc = bacc.Bacc(
        "TRN2",
        target_bir_lowering=False,
        debug=False,
        enable_asserts=False,
        num_devices=ncores,
    )

    at_d = nc.dram_tensor("at", [n, rpc], dtr, kind="ExternalInput")
    xt_d = nc.dram_tensor("xt", [f, n], dtr, kind="ExternalInput")
    wt_d = nc.dram_tensor("wt", [f, f], dtr, kind="ExternalInput")
    b_d = nc.dram_tensor("bias", [1, f], dt, kind="ExternalInput")
    ones_r_d = nc.dram_tensor("ones_r", [1, P], dt, kind="ExternalInput")
    ones_c_d = nc.dram_tensor("ones_c", [P, 1], dtr, kind="ExternalInput")
    out_d = nc.dram_tensor("out_t", [f, rpc], dt, kind="ExternalOutput")
    if debug_taps:
        tap_sqrt_d = nc.dram_tensor("tap_sqrt", [1, rpc], dt, kind="ExternalOutput")
        tap_dinv_d = nc.dram_tensor("tap_dinv", [P, n // P], dt, kind="ExternalOutput")
        tap_s_d = nc.dram_tensor("tap_s", [P, (n // P) * f], dt, kind="ExternalOutput")

    with tile.TileContext(nc) as tc, ExitStack() as ctx:
        cpool = ctx.enter_context(tc.tile_pool(name="const", bufs=1))
        wt_sb = cpool.tile([f, f], dtr, name="wt_sb")
        bias_sb = cpool.tile([1, f], dt, name="bias_sb")
        ones_r = cpool.tile([1, P], dt, name="ones_r")
        ones_c = cpool.tile([P, 1], dtr, name="ones_c")
        b_rep = cpool.tile([P, f], dt, name="b_rep")
        sup_all = cpool.tile([P, jt * f], dtr, name="sup_all")
        dinv_sb = cpool.tile([P, jt], dt, name="dinv_sb")
        dinv_l = cpool.tile([1, rpc], dt, name="dinv_l")
        dinv_rep = cpool.tile([P, rpc], dt, name="dinv_rep")
        out_sb = cpool.tile([P, rpc], dt, name="out_sb")

        dram = ctx.enter_context(tc.tile_pool(name="dram", bufs=1, space="DRAM"))
        ag_in = dram.tile([1, rpc], dt, name="ag_in")
        ag_out = dram.tile([ncores, rpc], dt, name="ag_out",
                           addr_space="Shared" if ncores > 4 else "Local")

        nc.gpsimd.dma_start(wt_sb[:], wt_d[:])
        nc.gpsimd.dma_start(bias_sb[:], b_d[:])
        nc.gpsimd.dma_start(ones_r[:], ones_r_d[:])
        nc.gpsimd.dma_start(ones_c[:], ones_c_d[:])

        apool_res = ctx.enter_context(tc.tile_pool(name="a_res", bufs=max(nres_pairs, 1)))
        apool_str = ctx.enter_context(tc.tile_pool(name="a_str", bufs=stream_pairs))
        apool_rld = ctx.enter_context(tc.tile_pool(name="a_rld", bufs=reload_pairs))
        xpool = ctx.enter_context(tc.tile_pool(name="xts", bufs=4))
        res_tiles = {}   # pair index -> tile [P, 2*rpc]

        def load_pair(pool, pj, tag, nm, eng=None):
            t = pool.tile([P, 2 * rpc], dtr, name=nm, tag=tag)
            src = at_d[pj * 2 * P:(pj + 1) * 2 * P, :].rearrange(
                "(h p) i -> p h i", p=P)
            (eng or nc.sync).dma_start(t[:], src)
            return t

        # ---- Phase A: support = x @ W.T + b, and degree row sums ----
        with (
            tc.tile_pool(name="psum_s", bufs=2, space="PSUM") as psum_s,
            tc.tile_pool(name="psum_r", bufs=1, space="PSUM") as psum_r,
        ):
            # bias broadcast via outer product: ones_r.T @ bias -> [P, f]
            pb = psum_s.tile([P, f], dt, name="pb", tag="pb")
            nc.tensor.matmul(pb[:], ones_r[:], bias_sb[:], start=True, stop=True)
            nc.vector.tensor_copy(b_rep[:], pb[:])

            for j in range(jt):
                xts = xpool.tile([f, f], dtr, name=f"xts{j}", tag="xts")
                nc.gpsimd.dma_start(xts[:], xt_d[:, j * f:(j + 1) * f])
                ps = psum_s.tile([P, f], dt, name=f"ps{j}", tag="ps")
                nc.tensor.matmul(ps[:], xts[:], wt_sb[:], start=True, stop=True)
                nc.vector.tensor_add(sup_all[:, j * f:(j + 1) * f], ps[:], b_rep[:])

            # degree: accumulate ones_c.T @ a_tile into [1, rpc]
            pr = psum_r.tile([1, rpc], dt, name="pr")
            for pj in range(pairs):
                if pj < nres_pairs:
                    a_t = load_pair(apool_res, pj, "ares", f"a{pj}")
                    res_tiles[pj] = a_t
                else:
                    a_t = load_pair(apool_str, pj, "astr", f"a{pj}")
                for half in range(2):
                    j = 2 * pj + half
                    for h in range(0, rpc, 512):
                        w = min(512, rpc - h)
                        nc.tensor.matmul(
                            pr[:, h:h + w], ones_c[:],
                            a_t[:, half * rpc + h:half * rpc + h + w],
                            start=(j == 0), stop=(j == jt - 1),
                        )

            nc.scalar.sqrt(dinv_l[:], pr[:])

        # Early-emit only the first reload_pairs reloads: they take fresh
        # pool slots (zero wait conditions) and fresh DMA sems, so they
        # stream during the AllGather window. The rest stay in phase D
        # (verified pipeline); emitting all of them early deadlocks.
        rld_tiles = {}
        for pj in range(nres_pairs, min(nres_pairs + reload_pairs, pairs)):
            rld_tiles[pj] = load_pair(apool_rld, pj, "arld", f"a2_{pj}")

        nc.vector.reciprocal(dinv_l[:], dinv_l[:])

        # row-scale broadcast dinv_rep = ones_r.T x dinv_l (local; overlaps AG)
        with tc.tile_pool(name="psum_d", bufs=1, space="PSUM") as psum_d:
            pd = psum_d.tile([f, rpc], dt, name="pd")
            for h in range(0, rpc, 512):
                w = min(512, rpc - h)
                nc.tensor.matmul(
                    pd[:, h:h + w], ones_r[:], dinv_l[:, h:h + w],
                    start=True, stop=True,
                )
            nc.vector.tensor_copy(dinv_rep[:], pd[:])

        # ---- Phase B: AllGather dinv across the cores ----
        nc.gpsimd.dma_start(ag_in[:], dinv_l[:])
        nc.gpsimd.collective_compute(
            "AllGather",
            mybir.AluOpType.bypass,
            replica_groups=[list(range(ncores))],
            ins=[ag_in.opt()],
            outs=[ag_out.opt()],
        )
        # dinv partition-major: dinv_sb[:, a*ncores + r0] = dinv[r0*rpc + a*P : +P]
        engs = [nc.scalar, nc.gpsimd]
        for a in range(per_rank):
            src = ag_out[:, a * P:(a + 1) * P].rearrange("r p -> p r")
            engs[a % len(engs)].dma_start(
                dinv_sb[:, a * ncores:(a + 1) * ncores], src)

        def dinv_col(j):  # column of dinv_sb holding dinv for j-tile j
            r0, a = j // per_rank, j % per_rank
            return a * ncores + r0

        # ---- Phase C: scale support columns: S[j, :] = dinv[j] * sup[j, :] ----
        # natural order: resident-pair columns (low j) come first
        for j in range(jt):
            sl = slice(j * f, (j + 1) * f)
            c = dinv_col(j)
            nc.vector.tensor_scalar_mul(sup_all[:, sl], sup_all[:, sl],
                                        dinv_sb[:, c:c + 1])

        if debug_taps:
            nc.scalar.dma_start(tap_sqrt_d[:], sqrt_t[:])
            nc.scalar.dma_start(tap_dinv_d[:], dinv_sb[:])
            nc.scalar.dma_start(tap_s_d[:], sup_all[:].bitcast(dt))

        # ---- Phase D: out.T = sum_j S[j].T @ a_hat.T[j] (accumulate over j) ----
        with tc.tile_pool(name="psum_o", bufs=1, space="PSUM") as psum_o:
            po = psum_o.tile([f, rpc], dt, name="po")
            order = list(range(pairs))
            for idx, pj in enumerate(order):
                if pj in res_tiles:
                    a_t = res_tiles[pj]
                elif pj in rld_tiles:
                    a_t = rld_tiles[pj]
                else:
                    a_t = load_pair(apool_rld, pj, "arld", f"a2_{pj}")
                for half in range(2):
                    j = 2 * pj + half
                    sl = slice(j * f, (j + 1) * f)
                    # start/stop are per psum REGION: first/last writer of
                    # each po[:, h] slice carries the flag
                    first = (idx == 0 and half == 0)
                    last = (idx == len(order) - 1 and half == 1)
                    for h in range(0, rpc, 512):
                        w = min(512, rpc - h)
                        nc.tensor.matmul(
                            po[:, h:h + w], sup_all[:, sl],
                            a_t[:, half * rpc + h:half * rpc + h + w],
                            start=first, stop=last,
                        )

            # ---- Phase E: out = dinv[i] * out ----
            nc.vector.tensor_mul(out_sb[:], po[:], dinv_rep[:])

        nc.scalar.dma_start(out_d[:], out_sb[:])

    nc.compile()
    return nc


_module_cache = {}


def _get_module():
    if "nc" not in _module_cache:
        nc = build_module()
        from concourse.bass_interp import get_hw_module

        nc.m = get_hw_module(nc.m)
        _module_cache["nc"] = nc
    return _module_cache["nc"]


def make_in_maps(x, adjacency, W, b, n=N, f=F, ncores=NCORES):
    rpc = n // ncores
    x = np.asarray(x, dtype=np.float32)
    adjacency = np.asarray(adjacency, dtype=np.float32)
    W = np.asarray(W, dtype=np.float32)
    b = np.asarray(b, dtype=np.float32)
    xt = np.ascontiguousarray(x.T)
    wt = np.ascontiguousarray(W.T)
    bias = np.ascontiguousarray(b.reshape(1, f))
    ones_r = np.ones((1, P), dtype=np.float32)
    ones_c = np.ones((P, 1), dtype=np.float32)
    in_maps = []
    for c in range(ncores):
        at = np.ascontiguousarray(adjacency[c * rpc:(c + 1) * rpc, :].T)
        # fold a_hat = A + I into the shard: global row c*rpc+i, column c*rpc+i
        at[c * rpc + np.arange(rpc), np.arange(rpc)] += 1.0
        in_maps.append({
            "at": at, "xt": xt, "wt": wt, "bias": bias,
            "ones_r": ones_r, "ones_c": ones_c,
        })
    return in_maps


def kernel(x, adjacency, W, b):
    from concourse.bass_utils import run_bass_kernel_spmd

    nc = _get_module()
    in_maps = make_in_maps(x, adjacency, W, b)
    res = run_bass_kernel_spmd(nc, in_maps, core_ids=list(range(NCORES)))
    out = np.empty((N, F), dtype=np.float32)
    for c in range(NCORES):
        out[c * RPC:(c + 1) * RPC, :] = res.results[c]["out_t"].T
    return out



# revision 5
# speedup vs baseline: 1.5530x; 1.5530x over previous
"""GCN layer (normalized adjacency aggregation) on 8 Trainium2 NeuronCores.

Algorithm (row-sharded, single bf16 residency + fp8 degree scout):
    a_hat = A + I  (identity folded into the shard on the host)
    deg[i] = sum_j a_hat[i, j]     -> fp8 scout copy, DoubleRow PE matmuls
    dinv = deg ** -0.5             -> sqrt + reciprocal, AllGather (4KB)
    sup = x @ W.T + b              -> computed redundantly per core (tiny)
    S = dinv[:, None] * sup
    out[i, :] = dinv[i] * (a_hat[i, :] @ S)

Two copies of the row block are shipped, both transposed (contraction dim j on
partitions) and tile-major:
  * a8  [8192, 1024] -> fp8 e4m3, streamed first; 64 DoubleRow ones-matmuls
    produce the degree row sums.  fp8 quantization error on the degree sum is
    ~2e-4 relative -- far below the 2e-2 gate.
  * ab  same data in bf16, streamed second and kept fully SBUF-resident
    (16 MB); the aggregation pass runs out of SBUF with zero HBM re-reads.
The point of the scout copy: the degree scan finishes after only 8 MB of DMA,
so the AllGather (a fixed ~25 us collective) runs concurrently with the bf16
stream instead of sitting exposed between the two PE passes.
"""

import numpy as np
from contextlib import ExitStack

N = 8192
F = 128
NCORES = 8
RPC = N // NCORES  # 1024 rows per core
P = 128            # partitions
JT = N // P        # 64 j-tiles of 128 columns
PAIRS = JT // 2    # 32 pair tiles of 256 columns

A8_BUFS = 6        # streaming pool for the fp8 scout tiles
XT_BUFS = 4


def build_module(use_scout=True):
    """Build and compile the SPMD Bass module (same program on every core)."""
    from concourse import bass, bacc, tile

    mybir = bass.mybir
    f32 = mybir.dt.float32
    bf16 = mybir.dt.bfloat16
    f8 = mybir.dt.float8e4

    nc = bacc.Bacc(
        "TRN2",
        target_bir_lowering=False,
        debug=False,
        enable_asserts=False,
        num_devices=NCORES,
    )

    a8_d = nc.dram_tensor("a8", [PAIRS * P, 2 * RPC], f8, kind="ExternalInput")
    ab_d = nc.dram_tensor("ab", [PAIRS * P, 2 * RPC], bf16, kind="ExternalInput")
    xt_d = nc.dram_tensor("xt", [F, N], bf16, kind="ExternalInput")
    wt_d = nc.dram_tensor("wt", [F, F], bf16, kind="ExternalInput")
    b_d = nc.dram_tensor("bias", [1, F], f32, kind="ExternalInput")
    ones_r_d = nc.dram_tensor("ones_r", [1, P], f32, kind="ExternalInput")
    ones8_d = nc.dram_tensor("ones8", [P, 64], f8, kind="ExternalInput")
    id64_d = nc.dram_tensor("id64", [64, 64], f32, kind="ExternalInput")
    out_d = nc.dram_tensor("out_t", [F, RPC], f32, kind="ExternalOutput")

    with tile.TileContext(nc) as tc, ExitStack() as ctx:
        cpool = ctx.enter_context(tc.tile_pool(name="const", bufs=1))
        wt_sb = cpool.tile([F, F], bf16, name="wt_sb")
        bias_sb = cpool.tile([1, F], f32, name="bias_sb")
        ones_r = cpool.tile([1, P], f32, name="ones_r")
        ones8 = cpool.tile([P, 2, 32], f8, name="ones8")
        id64 = cpool.tile([64, 64], f32, name="id64")
        b_rep = cpool.tile([P, F], f32, name="b_rep")
        sup_all = cpool.tile([P, JT * F], bf16, name="sup_all")
        s_all = cpool.tile([P, JT * F], bf16, name="s_all")
        dinv_l = cpool.tile([1, RPC], f32, name="dinv_l")
        dinv_sb = cpool.tile([P, JT], f32, name="dinv_sb")
        dinv_rep = cpool.tile([P, RPC], f32, name="dinv_rep")
        rvec = cpool.tile([64, P], f32, name="rvec")
        out_sb = cpool.tile([P, RPC], f32, name="out_sb")

        dram = ctx.enter_context(tc.tile_pool(name="dram", bufs=1, space="DRAM"))
        ag_in = dram.tile([1, RPC], f32, name="ag_in")
        ag_out = dram.tile([NCORES, RPC], f32, name="ag_out",
                           addr_space="Shared" if NCORES > 4 else "Local")

        nc.gpsimd.dma_start(wt_sb[:], wt_d[:])
        nc.gpsimd.dma_start(bias_sb[:], b_d[:])
        nc.gpsimd.dma_start(ones_r[:], ones_r_d[:])
        nc.gpsimd.dma_start(
            ones8[:], ones8_d[:].rearrange("p (a b) -> p a b", b=32))
        nc.gpsimd.dma_start(id64[:], id64_d[:])

        a8pool = ctx.enter_context(tc.tile_pool(name="a8p", bufs=A8_BUFS))
        abpool = ctx.enter_context(tc.tile_pool(name="abp", bufs=PAIRS))
        xpool = ctx.enter_context(tc.tile_pool(name="xts", bufs=XT_BUFS))

        # ---- Phase A: degree row sums off the fp8 scout stream ----
        with (
            tc.tile_pool(name="psum_r", bufs=1, space="PSUM") as psum_r,
            tc.tile_pool(name="psum_s", bufs=2, space="PSUM") as psum_s,
            tc.tile_pool(name="psum_b", bufs=1, space="PSUM") as psum_b,
        ):
            pr = psum_r.tile([32, RPC], f32, name="pr")
            for t in range(PAIRS):
                t8 = a8pool.tile([P, 2, RPC], f8, name=f"a8_{t}", tag="a8")
                nc.sync.dma_start(
                    t8[:],
                    a8_d[t * P:(t + 1) * P, :].rearrange(
                        "p (two i) -> p two i", two=2))
                for h in range(0, RPC, 512):
                    nc.tensor.matmul(
                        pr[:, h:h + 512], ones8[:], t8[:, :, h:h + 512],
                        start=(t == 0), stop=(t == PAIRS - 1),
                        perf_mode=mybir.MatmulPerfMode.DoubleRow,
                    )
            nc.scalar.sqrt(dinv_l[:], pr[0:1, :])

            # bias broadcast via outer product: ones_r.T @ bias -> [P, F]
            pb = psum_b.tile([P, F], f32, name="pb")
            nc.tensor.matmul(pb[:], ones_r[:], bias_sb[:], start=True, stop=True)
            nc.vector.tensor_copy(b_rep[:], pb[:])

            # support tiles: sup[j*F:(j+1)*F] = x_j @ W.T + b  (bf16, j on parts)
            for j in range(JT):
                xts = xpool.tile([F, F], bf16, name=f"xts{j}", tag="xts")
                nc.gpsimd.dma_start(xts[:], xt_d[:, j * F:(j + 1) * F])
                ps = psum_s.tile([P, F], f32, name=f"ps{j}", tag="ps")
                nc.tensor.matmul(ps[:], xts[:], wt_sb[:], start=True, stop=True)
                nc.vector.tensor_add(sup_all[:, j * F:(j + 1) * F], ps[:], b_rep[:])

        nc.vector.reciprocal(dinv_l[:], dinv_l[:])

        # ---- Phase B: AllGather dinv across the cores ----
        nc.gpsimd.dma_start(ag_in[:], dinv_l[:])
        nc.gpsimd.collective_compute(
            "AllGather",
            mybir.AluOpType.bypass,
            replica_groups=[list(range(NCORES))],
            ins=[ag_in.opt()],
            outs=[ag_out.opt()],
        )

        # ---- bf16 adjacency stream: queued behind the scout on the DMA rings,
        # fully resident in SBUF ----
        ab_tiles = []
        for t in range(PAIRS):
            tb = abpool.tile([P, 2, RPC], bf16, name=f"ab_{t}", tag="ab")
            nc.sync.dma_start(
                tb[:],
                ab_d[t * P:(t + 1) * P, :].rearrange(
                    "p (two i) -> p two i", two=2))
            ab_tiles.append(tb)

        with (
            tc.tile_pool(name="psum_d", bufs=1, space="PSUM") as psum_d,
            tc.tile_pool(name="psum_t", bufs=1, space="PSUM") as psum_t,
            tc.tile_pool(name="psum_o", bufs=1, space="PSUM") as psum_o,
        ):
            # local row-scale broadcast dinv_rep = ones_r.T x dinv_l (during AG)
            pd = psum_d.tile([F, RPC], f32, name="pd")
            for h in range(0, RPC, 512):
                nc.tensor.matmul(pd[:, h:h + 512], ones_r[:], dinv_l[:, h:h + 512],
                                 start=True, stop=True)
            nc.vector.tensor_copy(dinv_rep[:], pd[:])

            # post-AG: load dinv as [64, 128], PE-transpose to partition-major
            nc.scalar.dma_start(
                rvec[:], ag_out[:].rearrange("c (a b) -> (c a) b", b=P))
            pt = psum_t.tile([P, 64], f32, name="pt")
            nc.tensor.transpose(pt[:], rvec[:], id64[:])
            nc.vector.tensor_copy(dinv_sb[:], pt[:])

            # scale support columns: S[j] = dinv[j] * sup[j]  (vector+gpsimd)
            engs = [nc.vector, nc.gpsimd]
            for j in range(JT):
                sl = slice(j * F, (j + 1) * F)
                engs[j % 2].tensor_scalar_mul(
                    s_all[:, sl], sup_all[:, sl], dinv_sb[:, j:j + 1])

            # ---- Phase D: out.T = sum_j S[j].T @ a_hat.T[j] (SBUF-resident) ----
            po = psum_o.tile([F, RPC], f32, name="po")
            for pj in range(PAIRS):
                for half in range(2):
                    j = 2 * pj + half
                    sl = slice(j * F, (j + 1) * F)
                    for h in range(0, RPC, 512):
                        nc.tensor.matmul(
                            po[:, h:h + 512], s_all[:, sl],
                            ab_tiles[pj][:, half, h:h + 512],
                            start=(j == 0), stop=(j == JT - 1),
                        )

            # ---- Phase E: out = dinv[i] * out ----
            nc.vector.tensor_mul(out_sb[:], po[:], dinv_rep[:])

        nc.scalar.dma_start(out_d[:], out_sb[:])

    nc.compile()
    return nc


_module_cache = {}


def _get_module():
    if "nc" not in _module_cache:
        nc = build_module()
        from concourse.bass_interp import get_hw_module

        nc.m = get_hw_module(nc.m)
        _module_cache["nc"] = nc
    return _module_cache["nc"]


def make_in_maps(x, adjacency, W, b):
    import ml_dtypes

    bf16 = ml_dtypes.bfloat16
    f8 = ml_dtypes.float8_e4m3

    x = np.asarray(x, dtype=np.float32)
    adjacency = np.asarray(adjacency, dtype=np.float32)
    W = np.asarray(W, dtype=np.float32)
    b = np.asarray(b, dtype=np.float32)

    xtb = np.ascontiguousarray(x.T).astype(bf16)
    wtb = np.ascontiguousarray(W.T).astype(bf16)
    bias = np.ascontiguousarray(b.reshape(1, F))
    ones_r = np.ones((1, P), dtype=np.float32)
    ones8 = np.ones((P, 64), dtype=f8)
    id64 = np.eye(64, dtype=np.float32)

    in_maps = []
    for c in range(NCORES):
        at = np.ascontiguousarray(adjacency[c * RPC:(c + 1) * RPC, :].T)
        # fold a_hat = A + I into the shard: global row c*RPC+i, column c*RPC+i
        at[c * RPC + np.arange(RPC), np.arange(RPC)] += 1.0
        # tile-major: row t*128+p, col half*1024+i  <->  at[t*256+half*128+p, i]
        tiled = at.reshape(PAIRS, 2, P, RPC).transpose(0, 2, 1, 3)
        tiled = np.ascontiguousarray(tiled).reshape(PAIRS * P, 2 * RPC)
        in_maps.append({
            "a8": tiled.astype(f8), "ab": tiled.astype(bf16),
            "xt": xtb, "wt": wtb, "bias": bias,
            "ones_r": ones_r, "ones8": ones8, "id64": id64,
        })
    return in_maps


def kernel(x, adjacency, W, b):
    from concourse.bass_utils import run_bass_kernel_spmd

    nc = _get_module()
    in_maps = make_in_maps(x, adjacency, W, b)
    res = run_bass_kernel_spmd(nc, in_maps, core_ids=list(range(NCORES)))
    out = np.empty((N, F), dtype=np.float32)
    for c in range(NCORES):
        out[c * RPC:(c + 1) * RPC, :] = res.results[c]["out_t"].T
    return out


# revision 6
# speedup vs baseline: 1.5588x; 1.0037x over previous
"""GCN layer (normalized adjacency aggregation) on 8 Trainium2 NeuronCores.

Algorithm (row-sharded, single bf16 residency + fp8 degree scout):
    a_hat = A + I  (identity folded into the shard on the host)
    deg[i] = sum_j a_hat[i, j]     -> fp8 scout copy, DoubleRow PE matmuls
    dinv = deg ** -0.5             -> sqrt + reciprocal, AllGather (4KB)
    sup = x @ W.T + b              -> computed redundantly per core (tiny)
    S = dinv[:, None] * sup
    out[i, :] = dinv[i] * (a_hat[i, :] @ S)

Two copies of the row block are shipped, both transposed (contraction dim j on
partitions) and tile-major:
  * a8  fp8 e4m3, streamed first; 64 DoubleRow ones-matmuls produce the degree
    row sums.  fp8 quantization error on the degree sum is ~2e-4 relative.
  * ab  same data in bf16, streamed second and kept fully SBUF-resident
    (16 MB); the aggregation pass runs out of SBUF with zero HBM re-reads.
The scout copy exists so the degree scan (and with it the AllGather) finishes
after only 8 MB of DMA; the collective then overlaps the bf16 stream instead
of sitting exposed between the two PE passes.

Scheduling notes:
  * big streams alternate between the two HW DGE queues (SP + Activation);
    a single queue saturates at ~200 GB/s, two run near the 360 GB/s HBM cap.
  * the degree math runs on [32, 1024] (the DoubleRow stationary is 32 ones
    columns, so psum already holds 32 identical degree rows) -- a [1, 1024]
    vector op uses one DVE lane and costs ~6 us.
  * a dummy warm-up AllGather on garbage pays the collective's first-call
    setup during the scout phase.
"""

import numpy as np
from contextlib import ExitStack

N = 8192
F = 128
NCORES = 8
RPC = N // NCORES  # 1024 rows per core
P = 128            # partitions
JT = N // P        # 64 j-tiles of 128 columns
PAIRS = JT // 2    # 32 pair tiles of 256 columns

A8_BUFS = 12       # streaming pool for the fp8 scout tiles
XT_BUFS = 6
SUPB = 4           # support tiles batched per psum buffer / vector add


def build_module():
    """Build and compile the SPMD Bass module (same program on every core)."""
    from concourse import bass, bacc, tile

    mybir = bass.mybir
    f32 = mybir.dt.float32
    bf16 = mybir.dt.bfloat16
    f8 = mybir.dt.float8e4
    ACT = mybir.ActivationFunctionType

    nc = bacc.Bacc(
        "TRN2",
        target_bir_lowering=False,
        debug=False,
        enable_asserts=False,
        num_devices=NCORES,
    )

    a8_d = nc.dram_tensor("a8", [PAIRS * P, 2 * RPC], f8, kind="ExternalInput")
    ab_d = nc.dram_tensor("ab", [PAIRS * P, 2 * RPC], bf16, kind="ExternalInput")
    xt_d = nc.dram_tensor("xt", [F, N], bf16, kind="ExternalInput")
    wt_d = nc.dram_tensor("wt", [F, F], bf16, kind="ExternalInput")
    b4_d = nc.dram_tensor("bias4", [1, 4 * F], f32, kind="ExternalInput")
    ones_r_d = nc.dram_tensor("ones_r", [1, P], f32, kind="ExternalInput")
    ones8_d = nc.dram_tensor("ones8", [P, 64], f8, kind="ExternalInput")
    id64_d = nc.dram_tensor("id64", [64, 64], f32, kind="ExternalInput")
    out_d = nc.dram_tensor("out_t", [F, RPC], f32, kind="ExternalOutput")

    with tile.TileContext(nc) as tc, ExitStack() as ctx:
        cpool = ctx.enter_context(tc.tile_pool(name="const", bufs=1))
        wt_sb = cpool.tile([F, F], bf16, name="wt_sb")
        b4_sb = cpool.tile([1, 4 * F], f32, name="b4_sb")
        ones_r = cpool.tile([1, P], f32, name="ones_r")
        ones8 = cpool.tile([P, 2, 32], f8, name="ones8")
        id64 = cpool.tile([64, 64], f32, name="id64")
        b_rep = cpool.tile([P, 4 * F], f32, name="b_rep")
        sup_all = cpool.tile([P, JT * F], bf16, name="sup_all")
        s_all = cpool.tile([P, JT * F], bf16, name="s_all")
        d32 = cpool.tile([32, RPC], f32, name="d32")
        dinv_sb = cpool.tile([P, JT], f32, name="dinv_sb")
        dinv_rep = cpool.tile([P, RPC], f32, name="dinv_rep")
        rvec = cpool.tile([64, P], f32, name="rvec")
        out_sb = cpool.tile([P, RPC], f32, name="out_sb")

        dram = ctx.enter_context(tc.tile_pool(name="dram", bufs=1, space="DRAM"))
        ag_in = dram.tile([1, RPC], f32, name="ag_in")
        ag_out = dram.tile([NCORES, RPC], f32, name="ag_out", addr_space="Shared")
        dum_in = dram.tile([1, 64], f32, name="dum_in")
        dum_out = dram.tile([NCORES, 64], f32, name="dum_out", addr_space="Shared")

        nc.gpsimd.dma_start(wt_sb[:], wt_d[:])
        nc.gpsimd.dma_start(b4_sb[:], b4_d[:])
        nc.gpsimd.dma_start(ones_r[:], ones_r_d[:])
        nc.gpsimd.dma_start(
            ones8[:], ones8_d[:].rearrange("p (a b) -> p a b", b=32))
        nc.gpsimd.dma_start(id64[:], id64_d[:])

        # Warm-up collective on garbage: pays any first-call ring setup early,
        # while the scout stream is still running.
        with tc.high_priority():
            nc.gpsimd.collective_compute(
                "AllGather",
                mybir.AluOpType.bypass,
                replica_groups=[list(range(NCORES))],
                ins=[dum_in.opt()],
                outs=[dum_out.opt()],
            )

        a8pool = ctx.enter_context(tc.tile_pool(name="a8p", bufs=A8_BUFS))
        abpool = ctx.enter_context(tc.tile_pool(name="abp", bufs=PAIRS))
        xpool = ctx.enter_context(tc.tile_pool(name="xts", bufs=XT_BUFS))
        dma_engs = [nc.sync, nc.scalar]

        # ---- Phase A: degree row sums off the fp8 scout stream ----
        with (
            tc.tile_pool(name="psum_r", bufs=1, space="PSUM") as psum_r,
            tc.tile_pool(name="psum_s", bufs=3, space="PSUM") as psum_s,
            tc.tile_pool(name="psum_b", bufs=1, space="PSUM") as psum_b,
        ):
            pr = psum_r.tile([32, RPC], f32, name="pr")
            for t in range(PAIRS):
                t8 = a8pool.tile([P, 2, RPC], f8, name=f"a8_{t}", tag="a8")
                dma_engs[t % 2].dma_start(
                    t8[:],
                    a8_d[t * P:(t + 1) * P, :].rearrange(
                        "p (two i) -> p two i", two=2))
                for h in range(0, RPC, 512):
                    nc.tensor.matmul(
                        pr[:, h:h + 512], ones8[:], t8[:, :, h:h + 512],
                        start=(t == 0), stop=(t == PAIRS - 1),
                        perf_mode=mybir.MatmulPerfMode.DoubleRow,
                    )

            # dinv = deg ** -0.5 on 32 duplicate rows (32 DVE/ACT lanes)
            with tc.high_priority():
                nc.scalar.sqrt(d32[:], pr[:])
                nc.vector.reciprocal(d32[:], d32[:])
                nc.scalar.dma_start(ag_in[:], d32[0:1, :])
                nc.gpsimd.collective_compute(
                    "AllGather",
                    mybir.AluOpType.bypass,
                    replica_groups=[list(range(NCORES))],
                    ins=[ag_in.opt()],
                    outs=[ag_out.opt()],
                )

            # bias broadcast via outer product: ones_r.T @ bias4 -> [P, 4F]
            pb = psum_b.tile([P, 4 * F], f32, name="pb")
            nc.tensor.matmul(pb[:], ones_r[:], b4_sb[:], start=True, stop=True)
            nc.scalar.copy(b_rep[:], pb[:])

            # support tiles: sup[j] = x_j @ W.T + b, batched 4 tiles per psum
            for jq in range(JT // 4):
                ps = psum_s.tile([P, 4 * F], f32, name=f"ps{jq}", tag="ps")
                for k in range(4):
                    j = jq * 4 + k
                    xts = xpool.tile([F, F], bf16, name=f"xts{j}", tag="xts")
                    nc.gpsimd.dma_start(xts[:], xt_d[:, j * F:(j + 1) * F])
                    nc.tensor.matmul(ps[:, k * F:(k + 1) * F], xts[:], wt_sb[:],
                                     start=True, stop=True)
                sl = slice(jq * 4 * F, (jq + 1) * 4 * F)
                nc.vector.tensor_add(sup_all[:, sl], ps[:], b_rep[:])

        # ---- bf16 adjacency stream: queued behind the scout on both HW DGE
        # rings, fully resident in SBUF ----
        ab_tiles = []
        for t in range(PAIRS):
            tb = abpool.tile([P, 2, RPC], bf16, name=f"ab_{t}", tag="ab")
            dma_engs[t % 2].dma_start(
                tb[:],
                ab_d[t * P:(t + 1) * P, :].rearrange(
                    "p (two i) -> p two i", two=2))
            ab_tiles.append(tb)

        with (
            tc.tile_pool(name="psum_d", bufs=1, space="PSUM") as psum_d,
            tc.tile_pool(name="psum_t", bufs=1, space="PSUM") as psum_t,
            tc.tile_pool(name="psum_o", bufs=1, space="PSUM") as psum_o,
        ):
            # post-AG: load dinv as [64, 128], PE-transpose to partition-major
            pt = psum_t.tile([P, 64], f32, name="pt")
            with tc.high_priority():
                nc.gpsimd.dma_start(
                    rvec[:], ag_out[:].rearrange("c (a b) -> (c a) b", b=P))
                nc.tensor.transpose(pt[:], rvec[:], id64[:])
                nc.vector.tensor_copy(dinv_sb[:], pt[:])

            # scale support columns: S[j] = dinv[j] * sup[j] (vector + ACT)
            for j in range(JT):
                sl = slice(j * F, (j + 1) * F)
                if j % 2 == 0:
                    nc.vector.tensor_scalar_mul(
                        s_all[:, sl], sup_all[:, sl], dinv_sb[:, j:j + 1])
                else:
                    nc.scalar.activation(
                        s_all[:, sl], sup_all[:, sl], ACT.Copy,
                        scale=dinv_sb[:, j:j + 1])

            # ---- Phase D: out.T = sum_j S[j].T @ a_hat.T[j] (SBUF-resident) ----
            po = psum_o.tile([F, RPC], f32, name="po")
            for pj in range(PAIRS):
                for half in range(2):
                    j = 2 * pj + half
                    sl = slice(j * F, (j + 1) * F)
                    for h in range(0, RPC, 512):
                        nc.tensor.matmul(
                            po[:, h:h + 512], s_all[:, sl],
                            ab_tiles[pj][:, half, h:h + 512],
                            start=(j == 0), stop=(j == JT - 1),
                        )

            # local row-scale broadcast dinv_rep = ones_r.T x dinv (during AG)
            pd = psum_d.tile([F, RPC], f32, name="pd")
            for h in range(0, RPC, 512):
                nc.tensor.matmul(pd[:, h:h + 512], ones_r[:], d32[0:1, h:h + 512],
                                 start=True, stop=True)
            nc.vector.tensor_copy(dinv_rep[:], pd[:])

            # ---- Phase E: out = dinv[i] * out ----
            for h in range(0, RPC, 512):
                nc.vector.tensor_mul(out_sb[:, h:h + 512], po[:, h:h + 512],
                                     dinv_rep[:, h:h + 512])
                nc.scalar.dma_start(out_d[:, h:h + 512], out_sb[:, h:h + 512])

    nc.compile()
    return nc


_module_cache = {}


def _get_module():
    if "nc" not in _module_cache:
        nc = build_module()
        from concourse.bass_interp import get_hw_module

        nc.m = get_hw_module(nc.m)
        _module_cache["nc"] = nc
    return _module_cache["nc"]


def make_in_maps(x, adjacency, W, b):
    import ml_dtypes

    bf16 = ml_dtypes.bfloat16
    f8 = ml_dtypes.float8_e4m3

    x = np.asarray(x, dtype=np.float32)
    adjacency = np.asarray(adjacency, dtype=np.float32)
    W = np.asarray(W, dtype=np.float32)
    b = np.asarray(b, dtype=np.float32)

    xtb = np.ascontiguousarray(x.T).astype(bf16)
    wtb = np.ascontiguousarray(W.T).astype(bf16)
    bias4 = np.ascontiguousarray(np.tile(b, 4).reshape(1, 4 * F))
    ones_r = np.ones((1, P), dtype=np.float32)
    ones8 = np.ones((P, 64), dtype=f8)
    id64 = np.eye(64, dtype=np.float32)

    in_maps = []
    for c in range(NCORES):
        at = np.ascontiguousarray(adjacency[c * RPC:(c + 1) * RPC, :].T)
        # fold a_hat = A + I into the shard: global row c*RPC+i, column c*RPC+i
        at[c * RPC + np.arange(RPC), np.arange(RPC)] += 1.0
        # tile-major: row t*128+p, col half*1024+i  <->  at[t*256+half*128+p, i]
        tiled = at.reshape(PAIRS, 2, P, RPC).transpose(0, 2, 1, 3)
        tiled = np.ascontiguousarray(tiled).reshape(PAIRS * P, 2 * RPC)
        in_maps.append({
            "a8": tiled.astype(f8), "ab": tiled.astype(bf16),
            "xt": xtb, "wt": wtb, "bias4": bias4,
            "ones_r": ones_r, "ones8": ones8, "id64": id64,
        })
    return in_maps


def kernel(x, adjacency, W, b):
    from concourse.bass_utils import run_bass_kernel_spmd

    nc = _get_module()
    in_maps = make_in_maps(x, adjacency, W, b)
    res = run_bass_kernel_spmd(nc, in_maps, core_ids=list(range(NCORES)))
    out = np.empty((N, F), dtype=np.float32)
    for c in range(NCORES):
        out[c * RPC:(c + 1) * RPC, :] = res.results[c]["out_t"].T
    return out


# revision 7
# speedup vs baseline: 2.0251x; 1.2992x over previous
"""GCN layer (normalized adjacency aggregation) on 8 Trainium2 NeuronCores.

Algorithm (row-sharded, single bf16 residency + fp8 degree scout):
    a_hat = A + I  (identity folded into the shard on the host)
    deg[i] = sum_j a_hat[i, j]     -> fp8 scout copy, DoubleRow PE matmuls
    dinv = deg ** -0.5             -> sqrt + reciprocal, AllGather (4KB)
    sup = x @ W.T + b              -> computed redundantly per core (tiny)
    S = dinv[:, None] * sup
    out[i, :] = dinv[i] * (a_hat[i, :] @ S)

Two copies of the row block are shipped, both transposed (contraction dim j on
partitions) and tile-major:
  * a8  fp8 e4m3, streamed first; 64 DoubleRow ones-matmuls produce the degree
    row sums.  fp8 quantization error on the degree sum is ~2e-4 relative.
  * ab  same data in bf16, streamed second and kept fully SBUF-resident
    (16 MB); the aggregation pass runs out of SBUF with zero HBM re-reads.
The scout copy exists so the degree scan (and with it the AllGather) finishes
after only 8 MB of DMA; the collective then overlaps the bf16 stream instead
of sitting exposed between the two PE passes.

Scheduling notes:
  * big streams alternate between the two HW DGE queues (SP + Activation);
    a single queue saturates at ~200 GB/s, two run near the 360 GB/s HBM cap.
  * the degree math runs on [32, 1024] (the DoubleRow stationary is 32 ones
    columns, so psum already holds 32 identical degree rows) -- a [1, 1024]
    vector op uses one DVE lane and costs ~6 us.
  * a dummy warm-up AllGather on garbage pays the collective's first-call
    setup during the scout phase.
"""

import numpy as np
from contextlib import ExitStack

N = 8192
F = 128
NCORES = 8
RPC = N // NCORES  # 1024 rows per core
P = 128            # partitions
JT = N // P        # 64 j-tiles of 128 columns
PAIRS = JT // 2    # 32 pair tiles of 256 columns

A8_BUFS = 12       # streaming pool for the fp8 scout tiles
XT_BUFS = 6
SUPB = 4           # support tiles batched per psum buffer / vector add


def build_module():
    """Build and compile the SPMD Bass module (same program on every core)."""
    from concourse import bass, bacc, tile

    mybir = bass.mybir
    f32 = mybir.dt.float32
    bf16 = mybir.dt.bfloat16
    f8 = mybir.dt.float8e4
    ACT = mybir.ActivationFunctionType

    nc = bacc.Bacc(
        "TRN2",
        target_bir_lowering=False,
        debug=False,
        enable_asserts=False,
        num_devices=NCORES,
    )

    a8_d = nc.dram_tensor("a8", [PAIRS * P, 2 * RPC], f8, kind="ExternalInput")
    ab_d = nc.dram_tensor("ab", [PAIRS * P, 2 * RPC], bf16, kind="ExternalInput")
    xt_d = nc.dram_tensor("xt", [F, N], bf16, kind="ExternalInput")
    wt_d = nc.dram_tensor("wt", [F, F], bf16, kind="ExternalInput")
    b4_d = nc.dram_tensor("bias4", [1, 4 * F], f32, kind="ExternalInput")
    ones_r_d = nc.dram_tensor("ones_r", [1, P], f32, kind="ExternalInput")
    ones8_d = nc.dram_tensor("ones8", [P, 64], f8, kind="ExternalInput")
    id64_d = nc.dram_tensor("id64", [64, 64], f32, kind="ExternalInput")
    out_d = nc.dram_tensor("out_t", [F, RPC], f32, kind="ExternalOutput")

    with tile.TileContext(nc) as tc, ExitStack() as ctx:
        cpool = ctx.enter_context(tc.tile_pool(name="const", bufs=1))
        wt_sb = cpool.tile([F, F], bf16, name="wt_sb")
        b4_sb = cpool.tile([1, 4 * F], f32, name="b4_sb")
        ones_r = cpool.tile([1, P], f32, name="ones_r")
        ones8 = cpool.tile([P, 2, 32], f8, name="ones8")
        id64 = cpool.tile([64, 64], f32, name="id64")
        b_rep = cpool.tile([P, 4 * F], f32, name="b_rep")
        sup_all = cpool.tile([P, JT * F], bf16, name="sup_all")
        s_all = cpool.tile([P, JT * F], bf16, name="s_all")
        d32 = cpool.tile([32, RPC], f32, name="d32")
        dinv_sb = cpool.tile([P, JT], f32, name="dinv_sb")
        dinv_rep = cpool.tile([P, RPC], f32, name="dinv_rep")
        rvec = cpool.tile([64, P], f32, name="rvec")
        out_sb = cpool.tile([P, RPC], f32, name="out_sb")

        dram = ctx.enter_context(tc.tile_pool(name="dram", bufs=1, space="DRAM"))
        ag_in = dram.tile([1, RPC], f32, name="ag_in")
        ag_out = dram.tile([NCORES, RPC], f32, name="ag_out", addr_space="Shared")
        nc.gpsimd.dma_start(wt_sb[:], wt_d[:])
        nc.gpsimd.dma_start(b4_sb[:], b4_d[:])
        nc.gpsimd.dma_start(ones_r[:], ones_r_d[:])
        nc.gpsimd.dma_start(
            ones8[:], ones8_d[:].rearrange("p (a b) -> p a b", b=32))
        nc.gpsimd.dma_start(id64[:], id64_d[:])

        a8pool = ctx.enter_context(tc.tile_pool(name="a8p", bufs=A8_BUFS))
        abpool = ctx.enter_context(tc.tile_pool(name="abp", bufs=PAIRS))
        xpool = ctx.enter_context(tc.tile_pool(name="xts", bufs=XT_BUFS))
        dma_engs = [nc.sync, nc.scalar]

        # ---- Phase A: degree row sums off the fp8 scout stream ----
        with (
            tc.tile_pool(name="psum_r", bufs=1, space="PSUM") as psum_r,
            tc.tile_pool(name="psum_s", bufs=3, space="PSUM") as psum_s,
            tc.tile_pool(name="psum_b", bufs=1, space="PSUM") as psum_b,
        ):
            pr = psum_r.tile([32, RPC], f32, name="pr")
            for t in range(PAIRS):
                t8 = a8pool.tile([P, 2, RPC], f8, name=f"a8_{t}", tag="a8")
                dma_engs[t % 2].dma_start(
                    t8[:],
                    a8_d[t * P:(t + 1) * P, :].rearrange(
                        "p (two i) -> p two i", two=2))
                with tc.high_priority():
                    for h in range(0, RPC, 512):
                        nc.tensor.matmul(
                            pr[:, h:h + 512], ones8[:], t8[:, :, h:h + 512],
                            start=(t == 0), stop=(t == PAIRS - 1),
                            perf_mode=mybir.MatmulPerfMode.DoubleRow,
                        )

            # dinv = deg ** -0.5 on 32 duplicate rows (32 DVE/ACT lanes)
            with tc.high_priority():
                nc.scalar.sqrt(d32[:], pr[:])
                nc.vector.reciprocal(d32[:], d32[:])
                nc.gpsimd.dma_start(ag_in[:], d32[0:1, :])
                nc.gpsimd.collective_compute(
                    "AllGather",
                    mybir.AluOpType.bypass,
                    replica_groups=[list(range(NCORES))],
                    ins=[ag_in.opt()],
                    outs=[ag_out.opt()],
                )

            # bias broadcast via outer product: ones_r.T @ bias4 -> [P, 4F]
            pb = psum_b.tile([P, 4 * F], f32, name="pb")
            nc.tensor.matmul(pb[:], ones_r[:], b4_sb[:], start=True, stop=True)
            nc.scalar.copy(b_rep[:], pb[:])

            # support tiles: sup[j] = x_j @ W.T + b, batched 4 tiles per psum
            for jq in range(JT // 4):
                ps = psum_s.tile([P, 4 * F], f32, name=f"ps{jq}", tag="ps")
                for k in range(4):
                    j = jq * 4 + k
                    xts = xpool.tile([F, F], bf16, name=f"xts{j}", tag="xts")
                    dma_engs[j % 2].dma_start(xts[:], xt_d[:, j * F:(j + 1) * F])
                    nc.tensor.matmul(ps[:, k * F:(k + 1) * F], xts[:], wt_sb[:],
                                     start=True, stop=True)
                sl = slice(jq * 4 * F, (jq + 1) * 4 * F)
                nc.vector.tensor_add(sup_all[:, sl], ps[:], b_rep[:])

        # ---- bf16 adjacency stream: queued behind the scout on both HW DGE
        # rings, fully resident in SBUF ----
        ab_tiles = []
        for t in range(PAIRS):
            tb = abpool.tile([P, 2, RPC], bf16, name=f"ab_{t}", tag="ab")
            dma_engs[t % 2].dma_start(
                tb[:],
                ab_d[t * P:(t + 1) * P, :].rearrange(
                    "p (two i) -> p two i", two=2))
            ab_tiles.append(tb)

        with (
            tc.tile_pool(name="psum_d", bufs=1, space="PSUM") as psum_d,
            tc.tile_pool(name="psum_t", bufs=1, space="PSUM") as psum_t,
            tc.tile_pool(name="psum_o", bufs=1, space="PSUM") as psum_o,
        ):
            # post-AG: load dinv as [64, 128], PE-transpose to partition-major
            pt = psum_t.tile([P, 64], f32, name="pt")
            with tc.high_priority():
                nc.gpsimd.dma_start(
                    rvec[:], ag_out[:].rearrange("c (a b) -> (c a) b", b=P))
                nc.tensor.transpose(pt[:], rvec[:], id64[:])
                nc.vector.tensor_copy(dinv_sb[:], pt[:])

            # scale support columns: S[j] = dinv[j] * sup[j] (vector + ACT)
            for j in range(JT):
                sl = slice(j * F, (j + 1) * F)
                if j % 2 == 0:
                    nc.vector.tensor_scalar_mul(
                        s_all[:, sl], sup_all[:, sl], dinv_sb[:, j:j + 1])
                else:
                    nc.scalar.activation(
                        s_all[:, sl], sup_all[:, sl], ACT.Copy,
                        scale=dinv_sb[:, j:j + 1])

            # ---- Phase D: out.T = sum_j S[j].T @ a_hat.T[j] (SBUF-resident) ----
            po = psum_o.tile([F, RPC], f32, name="po")
            for pj in range(PAIRS):
                for half in range(2):
                    j = 2 * pj + half
                    sl = slice(j * F, (j + 1) * F)
                    for h in range(0, RPC, 512):
                        nc.tensor.matmul(
                            po[:, h:h + 512], s_all[:, sl],
                            ab_tiles[pj][:, half, h:h + 512],
                            start=(j == 0), stop=(j == JT - 1),
                        )

            # local row-scale broadcast dinv_rep = ones_r.T x dinv (during AG)
            pd = psum_d.tile([F, RPC], f32, name="pd")
            for h in range(0, RPC, 512):
                nc.tensor.matmul(pd[:, h:h + 512], ones_r[:], d32[0:1, h:h + 512],
                                 start=True, stop=True)
            nc.vector.tensor_copy(dinv_rep[:], pd[:])

            # ---- Phase E: out = dinv[i] * out ----
            for h in range(0, RPC, 512):
                nc.vector.tensor_mul(out_sb[:, h:h + 512], po[:, h:h + 512],
                                     dinv_rep[:, h:h + 512])
                nc.scalar.dma_start(out_d[:, h:h + 512], out_sb[:, h:h + 512])

    nc.compile()
    return nc


_module_cache = {}


def _get_module():
    if "nc" not in _module_cache:
        nc = build_module()
        from concourse.bass_interp import get_hw_module

        nc.m = get_hw_module(nc.m)
        _module_cache["nc"] = nc
    return _module_cache["nc"]


def make_in_maps(x, adjacency, W, b):
    import ml_dtypes

    bf16 = ml_dtypes.bfloat16
    f8 = ml_dtypes.float8_e4m3

    x = np.asarray(x, dtype=np.float32)
    adjacency = np.asarray(adjacency, dtype=np.float32)
    W = np.asarray(W, dtype=np.float32)
    b = np.asarray(b, dtype=np.float32)

    xtb = np.ascontiguousarray(x.T).astype(bf16)
    wtb = np.ascontiguousarray(W.T).astype(bf16)
    bias4 = np.ascontiguousarray(np.tile(b, 4).reshape(1, 4 * F))
    ones_r = np.ones((1, P), dtype=np.float32)
    ones8 = np.ones((P, 64), dtype=f8)
    id64 = np.eye(64, dtype=np.float32)

    in_maps = []
    for c in range(NCORES):
        at = np.ascontiguousarray(adjacency[c * RPC:(c + 1) * RPC, :].T)
        # fold a_hat = A + I into the shard: global row c*RPC+i, column c*RPC+i
        at[c * RPC + np.arange(RPC), np.arange(RPC)] += 1.0
        # tile-major: row t*128+p, col half*1024+i  <->  at[t*256+half*128+p, i]
        tiled = at.reshape(PAIRS, 2, P, RPC).transpose(0, 2, 1, 3)
        tiled = np.ascontiguousarray(tiled).reshape(PAIRS * P, 2 * RPC)
        in_maps.append({
            "a8": tiled.astype(f8), "ab": tiled.astype(bf16),
            "xt": xtb, "wt": wtb, "bias4": bias4,
            "ones_r": ones_r, "ones8": ones8, "id64": id64,
        })
    return in_maps


def kernel(x, adjacency, W, b):
    from concourse.bass_utils import run_bass_kernel_spmd

    nc = _get_module()
    in_maps = make_in_maps(x, adjacency, W, b)
    res = run_bass_kernel_spmd(nc, in_maps, core_ids=list(range(NCORES)))
    out = np.empty((N, F), dtype=np.float32)
    for c in range(NCORES):
        out[c * RPC:(c + 1) * RPC, :] = res.results[c]["out_t"].T
    return out
